# revision 2
# baseline (speedup 1.0000x reference)
"""Trainium2 Bass kernel for nn_CCL_Module (3x3 cost-volume softmax flow).

Reference computation (per batch):
  c1 = l2norm_C(feature1); wp = l2norm_C(feature2) zero-padded spatially.
  match_vol[d=(dh,dw)] = sum_C c1 * shift(wp, dh, dw)      (9 shifts, 3x3)
  p = softmax(10 * match_vol, over d)
  flow_w = sum_d p * dw ; flow_h = sum_d p * dh
  out = concat([flow_w, flow_h])  -> [B, 2, H, W]

Strategy (pure data parallel, one batch per NeuronCore, 8 cores):
  - SBUF layout: H=128 on partitions, free dims = (C=64, W).
  - dh shifts  -> three h-shifted copies of feature2 loaded by DMA.
  - dw shifts  -> free-dim AP offsets into w-padded tiles.
  - Raw (unnormalized) dots A_d = sum_C f1 * shift(f2) via DVE
    tensor_mul + strided tensor_reduce (reduce innermost = C).
  - L2 normalization folded into score scaling:
      score_d = 10 * A_d * rsqrt(|f1|^2) * rsqrt(|f2|^2 shifted)
  - Scores are bounded by |10| so softmax needs no max subtraction:
      flow = (sum_d w_d * exp(s_d)) / (sum_d exp(s_d))
"""

import numpy as np

B, C, H, W = 8, 64, 128, 128
N_CORES = 8
SOFTMAX_SCALE = 10.0

_CACHE = {}


def _build_program(repeat: int = 1, variant: str = "full"):
    import concourse.bass as bass
    import concourse.bacc as bacc
    import concourse.mybir as mybir
    from concourse.tile import TileContext
    from concourse.bass_utils import axon_active

    f32 = mybir.dt.float32
    nc = bacc.Bacc(
        "TRN2",
        target_bir_lowering=False,
        debug=not axon_active(),
        num_devices=N_CORES,
    )

    f1d = nc.declare_dram_parameter("feature1", [C, H, W], f32, isOutput=False)
    f2d = nc.declare_dram_parameter("feature2", [C, H, W], f32, isOutput=False)
    outd = nc.declare_dram_parameter("flow", [2, H, W], f32, isOutput=True)

    # DRAM views with h on the outer (partition) axis.
    f1v = f1d.rearrange("c h w -> h c w")
    f2v = f2d.rearrange("c h w -> h c w")
    outv = outd.rearrange("c h w -> h c w")

    # all-zero row used to zero-fill the dh edge partitions at load time
    zrow = nc.inline_tensor(np.zeros((1, C, W + 2), dtype=np.float32), name="zrow")

    with TileContext(nc) as tc:
        with tc.tile_pool(name="main", bufs=1) as pool:
          for _rep in range(repeat):
            # ---- input tiles ----
            xf1 = pool.tile([H, C, W], f32)          # f1, no padding
            # f2 with w padding (cols 0 and W+1), one tile per dh in {-1,0,1}.
            xf2_m = pool.tile([H, C, W + 2], f32)
            xf2_0 = pool.tile([H, C, W + 2], f32)
            xf2_p = pool.tile([H, C, W + 2], f32)

            nc.sync.dma_start(out=xf1[:, :, :], in_=f1v)
            # dh=0
            nc.sync.dma_start(out=xf2_0[:, :, 1 : W + 1], in_=f2v)
            # dh=-1: partition p holds f2 row p-1; row 0 is out of bounds -> 0
            nc.sync.dma_start(out=xf2_m[1:H, :, 1 : W + 1], in_=f2v[0 : H - 1])
            nc.sync.dma_start(out=xf2_m[0:1, :, :], in_=zrow[:])
            # dh=+1: partition p holds f2 row p+1; row H-1 out of bounds -> 0
            nc.sync.dma_start(out=xf2_p[0 : H - 1, :, 1 : W + 1], in_=f2v[1:H])
            nc.sync.dma_start(out=xf2_p[H - 1 : H, :, :], in_=zrow[:])

            # zero the w-pad columns so dw edge dots are exactly 0
            # (edge partitions already fully zeroed above; partition-0-based
            # memsets are legal for compute engines)
            for t in (xf2_m, xf2_0, xf2_p):
                nc.vector.memset(t[:, :, 0:1], 0.0)
                nc.vector.memset(t[:, :, W + 1 : W + 2], 0.0)

            xf2 = [xf2_m, xf2_0, xf2_p]

            # ---- raw correlation dots ----
            prod = pool.tile([H, C, W], f32)
            scoresA = pool.tile([H, 9, W], f32)     # A_d, d = dh*3+dw

            nmuls = 0 if variant == "loads" else 9
            for d in range(nmuls):
                dh, dw = d // 3 - 1, d % 3 - 1
                src = xf2[dh + 1][:, :, 1 + dw : 1 + dw + W]
                nc.vector.tensor_mul(prod[:, :, :], xf1[:, :, :], src)
                if variant == "muls":
                    continue
                # reduce over C (innermost after permute)
                nc.vector.tensor_reduce(
                    scoresA[:, d, :],
                    prod.rearrange("h c w -> h w c"),
                    axis=mybir.AxisListType.X,
                    op=mybir.AluOpType.add,
                )
            if variant in ("loads", "muls"):
                # consume every loaded tile so DCE can't drop the DMAs
                flows0 = pool.tile([H, 2, W], f32)
                nc.vector.tensor_add(flows0[:, 0, :], xf1[:, 0, :], xf2_m[:, 0, 0:W])
                nc.vector.tensor_add(flows0[:, 0, :], flows0[:, 0, :], xf2_0[:, 0, 0:W])
                nc.vector.tensor_add(flows0[:, 1, :], xf2_p[:, 0, 0:W], prod[:, 0, :])
                nc.sync.dma_start(out=outv, in_=flows0[:, :, :])
                continue

            # ---- norms ----
            r1sq = pool.tile([H, W], f32)
            r2m = pool.tile([H, W + 2], f32)  # |f2|^2 map, w-padded
            nc.vector.tensor_mul(prod[:, :, :], xf1[:, :, :], xf1[:, :, :])
            nc.vector.tensor_reduce(
                r1sq[:, :],
                prod.rearrange("h c w -> h w c"),
                axis=mybir.AxisListType.X,
                op=mybir.AluOpType.add,
            )
            f20 = xf2_0[:, :, 1 : W + 1]
            nc.vector.tensor_mul(prod[:, :, :], f20, f20)
            nc.vector.memset(r2m[:, 0:1], 1.0)
            nc.vector.memset(r2m[:, W + 1 : W + 2], 1.0)
            nc.vector.tensor_reduce(
                r2m[:, 1 : W + 1],
                prod.rearrange("h c w -> h w c"),
                axis=mybir.AxisListType.X,
                op=mybir.AluOpType.add,
            )

            # recip1 = 1/sqrt(r1sq), recip2 = 1/sqrt(r2m)
            recip1 = pool.tile([H, W], f32)
            recip2 = pool.tile([H, W + 2], f32)
            nc.scalar.sqrt(recip1[:, :], r1sq[:, :])
            nc.vector.reciprocal(recip1[:, :], recip1[:, :])
            nc.scalar.sqrt(recip2[:, :], r2m[:, :])
            nc.vector.reciprocal(recip2[:, :], recip2[:, :])

            # dh-shifted copies of recip2. Compute engines cannot address
            # partition-shifted APs, so shift across partitions via
            # SBUF->SBUF DMA. Edge rows clamp (their A is exactly 0).
            rec2_m = pool.tile([H, W + 2], f32)
            rec2_p = pool.tile([H, W + 2], f32)
            nc.sync.dma_start(out=rec2_m[1:H, :], in_=recip2[0 : H - 1, :])
            nc.sync.dma_start(out=rec2_m[0:1, :], in_=recip2[0:1, :])
            nc.sync.dma_start(out=rec2_p[0 : H - 1, :], in_=recip2[1:H, :])
            nc.sync.dma_start(out=rec2_p[H - 1 : H, :], in_=recip2[H - 1 : H, :])
            rec2 = [rec2_m, recip2, rec2_p]

            # ---- scores -> exp ----
            rmul = pool.tile([H, 9, W], f32)
            for d in range(9):
                dh, dw = d // 3 - 1, d % 3 - 1
                nc.vector.tensor_mul(
                    rmul[:, d, :], recip1[:, :], rec2[dh + 1][:, 1 + dw : 1 + dw + W]
                )
            expo = pool.tile([H, 9, W], f32)
            nc.vector.tensor_mul(rmul[:, :, :], rmul[:, :, :], scoresA[:, :, :])
            nc.scalar.activation(
                expo[:, :, :],
                rmul[:, :, :],
                mybir.ActivationFunctionType.Exp,
                scale=SOFTMAX_SCALE,
            )

            # ---- softmax-weighted displacement sums ----
            esum = pool.tile([H, W], f32)
            fwp = pool.tile([H, W], f32)
            fwm = pool.tile([H, W], f32)
            fhp = pool.tile([H, W], f32)
            fhm = pool.tile([H, W], f32)
            ex4 = expo.rearrange("h (a b) w -> h a b w", a=3)
            red = dict(axis=mybir.AxisListType.X, op=mybir.AluOpType.add)
            nc.vector.tensor_reduce(
                esum[:, :], expo.rearrange("h d w -> h w d"), **red
            )
            nc.vector.tensor_reduce(
                fwp[:, :], ex4[:, :, 2, :].rearrange("h a w -> h w a"), **red
            )
            nc.vector.tensor_reduce(
                fwm[:, :], ex4[:, :, 0, :].rearrange("h a w -> h w a"), **red
            )
            nc.vector.tensor_reduce(
                fhp[:, :], ex4[:, 2, :, :].rearrange("h b w -> h w b"), **red
            )
            nc.vector.tensor_reduce(
                fhm[:, :], ex4[:, 0, :, :].rearrange("h b w -> h w b"), **red
            )

            flows = pool.tile([H, 2, W], f32)
            nc.vector.reciprocal(esum[:, :], esum[:, :])
            nc.vector.tensor_sub(fwp[:, :], fwp[:, :], fwm[:, :])
            nc.vector.tensor_sub(fhp[:, :], fhp[:, :], fhm[:, :])
            nc.vector.tensor_mul(flows[:, 0, :], fwp[:, :], esum[:, :])
            nc.vector.tensor_mul(flows[:, 1, :], fhp[:, :], esum[:, :])

            nc.sync.dma_start(out=outv, in_=flows[:, :, :])

    nc.compile()
    return nc


def _make_in_maps(f1: np.ndarray, f2: np.ndarray):
    return [{"feature1": f1[b], "feature2": f2[b]} for b in range(N_CORES)]


def kernel(feature1: np.ndarray, feature2: np.ndarray) -> np.ndarray:
    from concourse import bass_utils

    if "nc" not in _CACHE:
        _CACHE["nc"] = _build_program()
    nc = _CACHE["nc"]

    f1 = np.ascontiguousarray(np.asarray(feature1, dtype=np.float32))
    f2 = np.ascontiguousarray(np.asarray(feature2, dtype=np.float32))
    in_maps = _make_in_maps(f1, f2)
    res = bass_utils.run_bass_kernel_spmd(nc, in_maps, list(range(N_CORES)))
    out = np.stack([res.results[b]["flow"] for b in range(N_CORES)], axis=0)
    return out.astype(np.float32)



# revision 9
# speedup vs baseline: 7.6111x; 7.6111x over previous
"""Trainium2 Bass kernel for nn_CCL_Module (3x3 cost-volume softmax flow).

Reference computation (per batch):
  c1 = l2norm_C(feature1); wp = l2norm_C(feature2) zero-padded spatially.
  match_vol[d=(dh,dw)] = sum_C c1 * shift(wp, dh, dw)      (9 shifts, 3x3)
  p = softmax(10 * match_vol, over d)
  flow_w = sum_d p * dw ; flow_h = sum_d p * dh
  out = concat([flow_w, flow_h])  -> [B, 2, H, W]

Strategy (pure data parallel, one batch per NeuronCore, 8 cores):
  - SBUF layout: partition p = b*64 + c  (c = channel, b = H-half), free
    dims are (row-within-half, w).  Both dh and dw shifts become plain
    free-dim offsets into an h/w-padded fp16 copy of feature2, and the
    DMA loads are fully contiguous per partition (the h-on-partition
    layout of the previous version needed partition-shifted loads that
    ran at ~20 GB/s).
  - Inputs are cast fp32->fp16 during the DMA (SWDGE cast) so the 9 big
    element-wise products run in the DVE 2x perf mode.  A +1-element
    shifted copy of the f2 tile keeps the dw=+-1 reads 4-byte aligned
    (2x mode requires it).
  - The channel reduction (sum over C=64 within each half) runs on the
    otherwise-idle TensorEngine: each 128-column chunk of the product is
    the stationary operand of a matmul against a constant [128,2] 0/1
    mask whose two columns select the two halves.  PSUM output lands as
    [w, (h2, half)] = A_d transposed, 512 B per partition per shift.
  - l2 normalization is folded into the softmax scores:
      score_d = 10 * A_d * rsqrt(|f1|^2) * rsqrt(|f2|^2 shifted)
    with |.|^2 maps reduced the same way from fp16 squares (ScalarE) and
    rsqrt computed as exp(-0.5*ln(x)) so Ln/Exp share one ACT table set.
    Out-of-image shifted positions are handled by zero rows in the f2
    tile (dh) and by zeroed edge rows of the w-shifted rsqrt maps (dw),
    which forces score=0 => exp(0)=1, exactly the reference behaviour
    for zero padding.
  - Scores are bounded by |10| so softmax needs no max subtraction:
      flow = (sum_d w_d * exp(s_d)) / (sum_d exp(s_d))
  - Output is produced in [w, (ch, h)] layout and written to a
    [2, W, H] DRAM tensor; the host transposes back to [2, H, W].
"""

import numpy as np

B, C, H, W = 8, 64, 128, 128
N_CORES = 8
SOFTMAX_SCALE = 10.0

NH = 64          # rows per H-half
N1 = NH * W      # 8192 free elems per partition for f1 / products
RT = 68          # f2 tile rows (rows 1..66 hold data, 0/67 are pad)
N2 = RT * W      # 8704

_CACHE = {}


def _build_program():
    import concourse.bass as bass
    import concourse.bacc as bacc
    import concourse.mybir as mybir
    from concourse.tile import TileContext
    from concourse.bass_utils import axon_active

    f32 = mybir.dt.float32
    f16 = mybir.dt.float16
    AF = mybir.ActivationFunctionType
    nc = bacc.Bacc(
        "TRN2",
        target_bir_lowering=False,
        debug=not axon_active(),
        num_devices=N_CORES,
    )

    f1d = nc.declare_dram_parameter("feature1", [C, H, W], f32, isOutput=False)
    f2d = nc.declare_dram_parameter("feature2", [C, H, W], f32, isOutput=False)
    # [W, 2, H]: transposed output, host permutes back to [2, H, W].
    outd = nc.declare_dram_parameter("flow", [W, 2, H], f32, isOutput=True)

    # partition p = b*64 + c ; free = (r, w), fully contiguous per partition
    f1flat = f1d.rearrange("c h w -> c (h w)")               # [64, 16384]
    f2flat = f2d.rearrange("c h w -> c (h w)")               # [64, 16384]
    outv = outd.rearrange("w ch (b h) -> w (ch b h)", b=2)   # [128, 256]

    mask_np = np.zeros((128, 2), dtype=np.float16)
    mask_np[:64, 0] = 1.0
    mask_np[64:, 1] = 1.0
    maskd = nc.inline_tensor(mask_np, name="halfmask")

    red = dict(axis=mybir.AxisListType.X, op=mybir.AluOpType.add)

    with TileContext(nc) as tc:
        with tc.tile_pool(name="main", bufs=1) as pool, \
             tc.tile_pool(name="prodp", bufs=3) as prodp, \
             tc.tile_pool(name="psA", bufs=4, space="PSUM") as psp, \
             tc.tile_pool(name="psR", bufs=1, space="PSUM") as psp1:

            f1t = pool.tile([128, N1], f16)
            f2t = pool.tile([128, N2], f16)
            f2s = pool.tile([128, N2], f16)
            mskt = pool.tile([128, 2], f16)

            nc.sync.dma_start(out=mskt[:, :], in_=maskd[:, :])

            # zero the dh pad rows (r=1 for half 0, r=66 for half 1) and the
            # corner-read rows 0/67; the loads below overwrite the data rows.
            nc.vector.memset(f2t[:, 0 : 2 * W], 0.0)
            nc.vector.memset(f2t[:, 66 * W : 68 * W], 0.0)

            # fp32 -> fp16 cast during DMA (SWDGE).  Row r in the tile holds
            # global row h = b*64 + r - 2.
            nc.gpsimd.dma_start(out=f1t[0:64, :], in_=f1flat[:, 0:N1])
            nc.gpsimd.dma_start(out=f1t[64:128, :], in_=f1flat[:, N1 : 2 * N1])
            nc.gpsimd.dma_start(
                out=f2t[0:64, 2 * W : 67 * W], in_=f2flat[:, 0 : 65 * W]
            )
            nc.gpsimd.dma_start(
                out=f2t[64:128, 1 * W : 66 * W], in_=f2flat[:, 63 * W : 128 * W]
            )

            # +1-element shifted copy so dw=+-1 product reads stay 4B-aligned
            # (DVE 2x mode); f2s[x] = f2t[x+1].
            nc.sync.dma_start(out=f2s[:, 0 : N2 - 1], in_=f2t[:, 1:N2])

            # |f1|^2 and |f2|^2 element squares (ScalarE, fp16 2x)
            sq1 = pool.tile([128, N1], f16)
            sq2 = pool.tile([128, N2], f16)
            nc.scalar.activation(sq1[:, :], f1t[:, :], AF.Square)
            nc.scalar.activation(sq2[:, W : 67 * W], f2t[:, W : 67 * W], AF.Square)

            # channel reduction of the squares on the TensorEngine:
            # psR[w, (t, half)] = sum_c sq[(c,half), t*W + w]
            psR1 = psp1.tile([128, 64, 2], f32, tag="psR1")
            psR2 = psp1.tile([128, 66, 2], f32, tag="psR2")
            for t in range(64):
                nc.tensor.matmul(
                    psR1[:, t, :],
                    lhsT=sq1[:, t * W : (t + 1) * W],
                    rhs=mskt[:, :],
                    start=(t == 0),
                    stop=(t == 63),
                )
            for t in range(66):
                nc.tensor.matmul(
                    psR2[:, t, :],
                    lhsT=sq2[:, (1 + t) * W : (2 + t) * W],
                    rhs=mskt[:, :],
                    start=(t == 0),
                    stop=(t == 65),
                )

            # rsqrt via exp(-0.5*ln(x)) -- Ln and Exp share one table set.
            # bias keeps ln finite on the zero pad rows (score there is 0).
            rinv1 = pool.tile([128, 64, 2], f32)
            rinv2 = pool.tile([128, 66, 2], f32)
            tl1 = pool.tile([128, 64, 2], f32)
            tl2 = pool.tile([128, 66, 2], f32)
            epsb = pool.tile([128, 1], f32)
            nc.vector.memset(epsb[:, :], 1e-20)
            nc.scalar.activation(tl1[:, :, :], psR1[:, :, :], AF.Ln, bias=epsb[:, :])
            nc.scalar.activation(rinv1[:, :, :], tl1[:, :, :], AF.Exp, scale=-0.5)
            nc.scalar.activation(tl2[:, :, :], psR2[:, :, :], AF.Ln, bias=epsb[:, :])
            nc.scalar.activation(rinv2[:, :, :], tl2[:, :, :], AF.Exp, scale=-0.5)

            # w-shifted copies of rinv2 (partition shift via SBUF->SBUF DMA).
            # Edge partitions stay 0 => score 0 => exp(0)=1, matching the
            # reference zero padding exactly.
            rec2m = pool.tile([128, 66, 2], f32)
            rec2p = pool.tile([128, 66, 2], f32)
            nc.vector.memset(rec2m[:, :, :], 0.0)
            nc.vector.memset(rec2p[:, :, :], 0.0)
            nc.sync.dma_start(out=rec2m[1:128, :, :], in_=rinv2[0:127, :, :])
            nc.sync.dma_start(out=rec2p[0:127, :, :], in_=rinv2[1:128, :, :])
            rsel = {-1: rec2m, 0: rinv2, 1: rec2p}

            # rr_d = rinv1 * rinv2(shifted) on GpSimd (parallel to DVE/PE)
            rrs = {}
            for d in range(9):
                dh, dw = d // 3 - 1, d % 3 - 1
                rr = pool.tile([128, 64, 2], f32, tag=f"rr{d}")
                nc.gpsimd.tensor_mul(
                    rr[:, :, :],
                    rinv1[:, :, :],
                    rsel[dw][:, 1 + dh : 65 + dh, :],
                )
                rrs[d] = rr

            # main loop: product (DVE 2x) -> mask-matmul reduction (PE)
            # -> score = A * rr (DVE, evacuates PSUM).
            # dw=0 shifts first: they don't wait on the f2s copy.
            s_all = pool.tile([128, 9, 64, 2], f32)
            for d in (1, 4, 7, 0, 2, 3, 5, 6, 8):
                dh, dw = d // 3 - 1, d % 3 - 1
                if dw == 0:
                    src, base = f2t, (2 + dh) * W
                else:
                    src, base = f2s, (2 + dh) * W + dw - 1
                prod = prodp.tile([128, N1], f16, tag="prod")
                nc.vector.tensor_mul(
                    prod[:, :], f1t[:, :], src[:, base : base + N1]
                )
                psA = psp.tile([128, 64, 2], f32, tag="psA")
                for t in range(64):
                    nc.tensor.matmul(
                        psA[:, t, :],
                        lhsT=prod[:, t * W : (t + 1) * W],
                        rhs=mskt[:, :],
                        start=(t == 0),
                        stop=(t == 63),
                    )
                nc.vector.tensor_mul(
                    s_all[:, d, :, :], psA[:, :, :], rrs[d][:, :, :]
                )

            # softmax-weighted displacement sums (no max subtraction needed:
            # |score| <= 10)
            e_all = pool.tile([128, 9, 64, 2], f32)
            nc.scalar.activation(
                e_all[:, :, :, :], s_all[:, :, :, :], AF.Exp, scale=SOFTMAX_SCALE
            )

            # reduce views are permuted so the outputs come out (b, h2)-major,
            # matching the DRAM layout; reduced axis is innermost (X).
            esum = pool.tile([128, 2, 64], f32)
            fwm = pool.tile([128, 2, 64], f32)
            fwp = pool.tile([128, 2, 64], f32)
            fhm = pool.tile([128, 2, 64], f32)
            fhp = pool.tile([128, 2, 64], f32)
            nc.vector.tensor_reduce(
                esum[:, :, :], e_all.rearrange("p d t m -> p m t d"), **red
            )
            ec = e_all.rearrange("p (a c) t m -> p c m t a", a=3)
            ea = e_all.rearrange("p (a c) t m -> p a m t c", a=3)
            nc.vector.tensor_reduce(fwm[:, :, :], ec[:, 0], **red)
            nc.vector.tensor_reduce(fwp[:, :, :], ec[:, 2], **red)
            nc.vector.tensor_reduce(fhm[:, :, :], ea[:, 0], **red)
            nc.vector.tensor_reduce(fhp[:, :, :], ea[:, 2], **red)

            resum = pool.tile([128, 2, 64], f32)
            nc.vector.reciprocal(resum[:, :, :], esum[:, :, :])
            nc.vector.tensor_sub(fwp[:, :, :], fwp[:, :, :], fwm[:, :, :])
            nc.vector.tensor_sub(fhp[:, :, :], fhp[:, :, :], fhm[:, :, :])

            # flows stored (w, ch, b, h2) so the DRAM write is contiguous
            flows = pool.tile([128, 2, 2, 64], f32)
            nc.vector.tensor_mul(flows[:, 0], fwp[:, :, :], resum[:, :, :])
            nc.vector.tensor_mul(flows[:, 1], fhp[:, :, :], resum[:, :, :])

            nc.sync.dma_start(out=outv, in_=flows[:, :, :, :])

    nc.compile()
    return nc


def _make_in_maps(f1: np.ndarray, f2: np.ndarray):
    return [{"feature1": f1[b], "feature2": f2[b]} for b in range(N_CORES)]


def kernel(feature1: np.ndarray, feature2: np.ndarray) -> np.ndarray:
    from concourse import bass_utils

    if "nc" not in _CACHE:
        _CACHE["nc"] = _build_program()
    nc = _CACHE["nc"]

    f1 = np.ascontiguousarray(np.asarray(feature1, dtype=np.float32))
    f2 = np.ascontiguousarray(np.asarray(feature2, dtype=np.float32))
    in_maps = _make_in_maps(f1, f2)
    res = bass_utils.run_bass_kernel_spmd(nc, in_maps, list(range(N_CORES)))
    # flow comes back [W, 2, H]; permute to [2, H, W]
    out = np.stack(
        [res.results[b]["flow"].transpose(1, 2, 0) for b in range(N_CORES)], axis=0
    )
    return np.ascontiguousarray(out.astype(np.float32))


# revision 10
# speedup vs baseline: 7.6639x; 1.0069x over previous
"""Trainium2 Bass kernel for nn_CCL_Module (3x3 cost-volume softmax flow).

Reference computation (per batch):
  c1 = l2norm_C(feature1); wp = l2norm_C(feature2) zero-padded spatially.
  match_vol[d=(dh,dw)] = sum_C c1 * shift(wp, dh, dw)      (9 shifts, 3x3)
  p = softmax(10 * match_vol, over d)
  flow_w = sum_d p * dw ; flow_h = sum_d p * dh
  out = concat([flow_w, flow_h])  -> [B, 2, H, W]

Strategy (pure data parallel, one batch per NeuronCore, 8 cores):
  - SBUF layout: partition p = b*64 + c  (c = channel, b = H-half), free
    dims are (row-within-half, w).  Both dh and dw shifts become plain
    free-dim offsets into an h/w-padded fp16 copy of feature2, and the
    DMA loads are fully contiguous per partition.
  - Inputs are cast fp32->fp16 during the DMA (SWDGE cast) so the 9 big
    element-wise products run in the DVE 2x perf mode.  A +1-element
    shifted copy of the f2 tile keeps the dw=+-1 reads 4-byte aligned.
  - The channel reduction (sum over C=64 within each half) runs on the
    otherwise-idle TensorEngine: each 128-column chunk of the product is
    the stationary operand of a matmul against a constant [128,2] 0/1
    mask whose columns select the halves.  PSUM output lands as
    [w, (h2, half)] = A_d transposed.  Three shifts share one PSUM bank
    (every matmul is its own start/stop group writing disjoint columns).
  - Everything is split into two row-half chunks so the DVE product
    chain starts as soon as the first half of the inputs has landed.
  - l2 normalization is folded into the softmax scores:
      score_d = 10 * A_d * rsqrt(|f1|^2) * rsqrt(|f2|^2 shifted)
    with rsqrt computed as exp(-0.5*ln(x)) (one ACT table set).
    Out-of-image shifted positions: zero rows in the f2 tile (dh) and
    zeroed edge rows of the w-shifted rsqrt maps (dw) force score=0 =>
    exp(0)=1, exactly the reference zero-padding behaviour.
  - Scores are bounded by |10| so softmax needs no max subtraction.
  - Output is written as [W, 2, H]; the host permutes back to [2, H, W].
"""

import numpy as np

B, C, H, W = 8, 64, 128, 128
N_CORES = 8
SOFTMAX_SCALE = 10.0

NH = 64          # rows per H-half
N1 = NH * W      # 8192 free elems per partition for f1 / products
RT = 68          # f2 tile rows (rows 1..66 hold data, 0/67 are pad)
N2 = RT * W      # 8704
HC = 32          # rows per chunk (half of NH)
NC1 = HC * W     # 4096, product chunk size

_CACHE = {}


def _build_program():
    import concourse.bass as bass
    import concourse.bacc as bacc
    import concourse.mybir as mybir
    from concourse.tile import TileContext
    from concourse.bass_utils import axon_active

    f32 = mybir.dt.float32
    f16 = mybir.dt.float16
    AF = mybir.ActivationFunctionType
    nc = bacc.Bacc(
        "TRN2",
        target_bir_lowering=False,
        debug=not axon_active(),
        num_devices=N_CORES,
    )

    f1d = nc.declare_dram_parameter("feature1", [C, H, W], f32, isOutput=False)
    f2d = nc.declare_dram_parameter("feature2", [C, H, W], f32, isOutput=False)
    # [W, 2, H]: transposed output, host permutes back to [2, H, W].
    outd = nc.declare_dram_parameter("flow", [W, 2, H], f32, isOutput=True)

    f1flat = f1d.rearrange("c h w -> c (h w)")               # [64, 16384]
    f2flat = f2d.rearrange("c h w -> c (h w)")               # [64, 16384]
    outv = outd.rearrange("w ch (b h) -> w (ch b h)", b=2)   # [128, 256]

    mask_np = np.zeros((128, 2), dtype=np.float16)
    mask_np[:64, 0] = 1.0
    mask_np[64:, 1] = 1.0
    maskd = nc.inline_tensor(mask_np, name="halfmask")

    red = dict(axis=mybir.AxisListType.X, op=mybir.AluOpType.add)
    DS = (1, 4, 7, 0, 2, 3, 5, 6, 8)  # dw=0 shifts first (no f2s needed)

    with TileContext(nc) as tc:
        with tc.tile_pool(name="main", bufs=1) as pool, \
             tc.tile_pool(name="prodp", bufs=3) as prodp, \
             tc.tile_pool(name="psum", bufs=1, space="PSUM") as psp:

            f1t = pool.tile([128, N1], f16)
            f2t = pool.tile([128, N2], f16)
            f2s = pool.tile([128, N2], f16)
            mskt = pool.tile([128, 2], f16)

            nc.sync.dma_start(out=mskt[:, :], in_=maskd[:, :])

            # zero the dh pad rows (r=1 half 0 / r=66 half 1) and the
            # corner-read rows 0/67; loads overwrite the data rows.
            nc.vector.memset(f2t[:, 0 : 2 * W], 0.0)
            nc.vector.memset(f2t[:, 66 * W : 68 * W], 0.0)

            # fp32 -> fp16 cast loads (SWDGE), chunked so the first-half
            # products can start before the second half lands.
            # f2 tile row r holds global row h = b*64 + r - 2.
            # chunk 1: f1 rows 0..31 per half, f2 tile rows <= 36
            nc.gpsimd.dma_start(out=f1t[0:64, 0:NC1], in_=f1flat[:, 0:NC1])
            nc.gpsimd.dma_start(
                out=f1t[64:128, 0:NC1], in_=f1flat[:, N1 : N1 + NC1]
            )
            # b=0: h = 0..34 -> r = 2..36 ; b=1: h = 63..98 -> r = 1..36
            nc.gpsimd.dma_start(
                out=f2t[0:64, 2 * W : 37 * W], in_=f2flat[:, 0 : 35 * W]
            )
            nc.gpsimd.dma_start(
                out=f2t[64:128, 1 * W : 37 * W], in_=f2flat[:, 63 * W : 99 * W]
            )
            # chunk 2
            nc.gpsimd.dma_start(
                out=f1t[0:64, NC1:N1], in_=f1flat[:, NC1:N1]
            )
            nc.gpsimd.dma_start(
                out=f1t[64:128, NC1:N1], in_=f1flat[:, N1 + NC1 : 2 * N1]
            )
            # b=0: h = 35..64 -> r = 37..66 ; b=1: h = 99..127 -> r = 37..65
            nc.gpsimd.dma_start(
                out=f2t[0:64, 37 * W : 67 * W], in_=f2flat[:, 35 * W : 65 * W]
            )
            nc.gpsimd.dma_start(
                out=f2t[64:128, 37 * W : 66 * W], in_=f2flat[:, 99 * W : 128 * W]
            )

            # +1-element shifted copy (4B alignment for dw=+-1), two HWDGE
            # rings in parallel; f2s[x] = f2t[x+1].
            nc.sync.dma_start(out=f2s[:, 0 : 37 * W], in_=f2t[:, 1 : 37 * W + 1])
            nc.scalar.dma_start(
                out=f2s[:, 37 * W : N2 - 1], in_=f2t[:, 37 * W + 1 : N2]
            )

            # |f1|^2, |f2|^2 squares (ScalarE), chunked like the loads
            sq1 = pool.tile([128, N1], f16)
            sq2 = pool.tile([128, N2], f16)
            nc.scalar.activation(sq1[:, 0:NC1], f1t[:, 0:NC1], AF.Square)
            nc.scalar.activation(
                sq2[:, W : 37 * W], f2t[:, W : 37 * W], AF.Square
            )
            nc.scalar.activation(sq1[:, NC1:N1], f1t[:, NC1:N1], AF.Square)
            nc.scalar.activation(
                sq2[:, 37 * W : 67 * W], f2t[:, 37 * W : 67 * W], AF.Square
            )

            # channel reduction of the squares on the TensorEngine:
            # psR[w, (t, half)] = sum_c sq[(c,half), t*W + w].
            # R1 and R2 share one PSUM bank (disjoint columns).
            psR = psp.tile([128, 2, 66, 2], f32, tag="psR")
            psR1, psR2 = psR[:, 0, 0:64, :], psR[:, 1, :, :]
            for t in range(64):
                nc.tensor.matmul(
                    psR1[:, t, :],
                    lhsT=sq1[:, t * W : (t + 1) * W],
                    rhs=mskt[:, :],
                )
            for t in range(66):
                nc.tensor.matmul(
                    psR2[:, t, :],
                    lhsT=sq2[:, (1 + t) * W : (2 + t) * W],
                    rhs=mskt[:, :],
                )

            # rsqrt via exp(-0.5*ln(x)); bias keeps ln finite on zero rows
            rinv1 = pool.tile([128, 64, 2], f32)
            rinv2 = pool.tile([128, 66, 2], f32)
            tl1 = pool.tile([128, 64, 2], f32)
            tl2 = pool.tile([128, 66, 2], f32)
            epsb = pool.tile([128, 1], f32)
            nc.vector.memset(epsb[:, :], 1e-20)
            nc.scalar.activation(tl1[:, :, :], psR1[:, :, :], AF.Ln, bias=epsb[:, :])
            nc.scalar.activation(rinv1[:, :, :], tl1[:, :, :], AF.Exp, scale=-0.5)
            nc.scalar.activation(tl2[:, :, :], psR2[:, :, :], AF.Ln, bias=epsb[:, :])
            nc.scalar.activation(rinv2[:, :, :], tl2[:, :, :], AF.Exp, scale=-0.5)

            # w-shifted copies of rinv2 (partition shift via SBUF->SBUF DMA);
            # edge partitions stay 0 => score 0 => exp(0)=1 (reference pad).
            rec2m = pool.tile([128, 66, 2], f32)
            rec2p = pool.tile([128, 66, 2], f32)
            nc.vector.memset(rec2m[:, :, :], 0.0)
            nc.vector.memset(rec2p[:, :, :], 0.0)
            nc.sync.dma_start(out=rec2m[1:128, :, :], in_=rinv2[0:127, :, :])
            nc.sync.dma_start(out=rec2p[0:127, :, :], in_=rinv2[1:128, :, :])
            rsel = {-1: rec2m, 0: rinv2, 1: rec2p}

            # rr_d = rinv1 * rinv2(shifted) on GpSimd (parallel engine)
            rrs = {}
            for d in DS:
                dh, dw = d // 3 - 1, d % 3 - 1
                rr = pool.tile([128, 64, 2], f32, tag=f"rr{d}")
                nc.gpsimd.tensor_mul(
                    rr[:, :, :], rinv1[:, :, :], rsel[dw][:, 1 + dh : 65 + dh, :]
                )
                rrs[d] = rr

            # 3 shifts share one PSUM bank; every matmul is its own group
            psA = {}
            for k in range(3):
                pa = psp.tile([128, 3, 64, 2], f32, tag=f"psA{k}")
                for j in range(3):
                    psA[DS[k * 3 + j]] = pa[:, j]

            def product_chunk(d, lo, hi):
                """prod[:, lo*W:hi*W] for shift d, then its PE reduction."""
                dh, dw = d // 3 - 1, d % 3 - 1
                if dw == 0:
                    src, base = f2t, (2 + dh) * W
                else:
                    src, base = f2s, (2 + dh) * W + dw - 1
                n = (hi - lo) * W
                prod = prodp.tile([128, NC1], f16, tag="prod")
                nc.vector.tensor_mul(
                    prod[:, 0:n],
                    f1t[:, lo * W : hi * W],
                    src[:, base + lo * W : base + hi * W],
                )
                for t in range(lo, hi):
                    nc.tensor.matmul(
                        psA[d][:, t, :],
                        lhsT=prod[:, (t - lo) * W : (t - lo + 1) * W],
                        rhs=mskt[:, :],
                    )

            s_all = pool.tile([128, 9, 64, 2], f32)
            e_all = pool.tile([128, 9, 64, 2], f32)
            for d in DS:
                product_chunk(d, 0, HC)
            for d in DS:
                product_chunk(d, HC, NH)
                nc.vector.tensor_mul(
                    s_all[:, d, :, :], psA[d][:, :, :], rrs[d][:, :, :]
                )
                nc.scalar.activation(
                    e_all[:, d, :, :], s_all[:, d, :, :], AF.Exp,
                    scale=SOFTMAX_SCALE,
                )

            # softmax-weighted displacement sums; outputs (b, h2)-major
            esum = pool.tile([128, 2, 64], f32)
            fwm = pool.tile([128, 2, 64], f32)
            fwp = pool.tile([128, 2, 64], f32)
            fhm = pool.tile([128, 2, 64], f32)
            fhp = pool.tile([128, 2, 64], f32)
            nc.vector.tensor_reduce(
                esum[:, :, :], e_all.rearrange("p d t m -> p m t d"), **red
            )
            ec = e_all.rearrange("p (a c) t m -> p c m t a", a=3)
            ea = e_all.rearrange("p (a c) t m -> p a m t c", a=3)
            nc.vector.tensor_reduce(fwm[:, :, :], ec[:, 0], **red)
            nc.vector.tensor_reduce(fwp[:, :, :], ec[:, 2], **red)
            nc.vector.tensor_reduce(fhm[:, :, :], ea[:, 0], **red)
            nc.vector.tensor_reduce(fhp[:, :, :], ea[:, 2], **red)

            resum = pool.tile([128, 2, 64], f32)
            nc.vector.reciprocal(resum[:, :, :], esum[:, :, :])
            nc.vector.tensor_sub(fwp[:, :, :], fwp[:, :, :], fwm[:, :, :])
            nc.vector.tensor_sub(fhp[:, :, :], fhp[:, :, :], fhm[:, :, :])

            # flows stored (w, ch, b, h2) so the DRAM write is contiguous
            flows = pool.tile([128, 2, 2, 64], f32)
            nc.vector.tensor_mul(flows[:, 0], fwp[:, :, :], resum[:, :, :])
            nc.vector.tensor_mul(flows[:, 1], fhp[:, :, :], resum[:, :, :])

            nc.sync.dma_start(out=outv, in_=flows[:, :, :, :])

    nc.compile()
    return nc


def _make_in_maps(f1: np.ndarray, f2: np.ndarray):
    return [{"feature1": f1[b], "feature2": f2[b]} for b in range(N_CORES)]


def kernel(feature1: np.ndarray, feature2: np.ndarray) -> np.ndarray:
    from concourse import bass_utils

    if "nc" not in _CACHE:
        _CACHE["nc"] = _build_program()
    nc = _CACHE["nc"]

    f1 = np.ascontiguousarray(np.asarray(feature1, dtype=np.float32))
    f2 = np.ascontiguousarray(np.asarray(feature2, dtype=np.float32))
    in_maps = _make_in_maps(f1, f2)
    res = bass_utils.run_bass_kernel_spmd(nc, in_maps, list(range(N_CORES)))
    # flow comes back [W, 2, H]; permute to [2, H, W]
    out = np.stack(
        [res.results[b]["flow"].transpose(1, 2, 0) for b in range(N_CORES)], axis=0
    )
    return np.ascontiguousarray(out.astype(np.float32))


# revision 13
# speedup vs baseline: 7.7546x; 1.0118x over previous
"""Trainium2 Bass kernel for nn_CCL_Module (3x3 cost-volume softmax flow).

Reference computation (per batch):
  c1 = l2norm_C(feature1); wp = l2norm_C(feature2) zero-padded spatially.
  match_vol[d=(dh,dw)] = sum_C c1 * shift(wp, dh, dw)      (9 shifts, 3x3)
  p = softmax(10 * match_vol, over d)
  flow_w = sum_d p * dw ; flow_h = sum_d p * dh
  out = concat([flow_w, flow_h])  -> [B, 2, H, W]

Strategy (pure data parallel, one batch per NeuronCore, 8 cores):
  - SBUF layout: partition p = b*64 + c  (c = channel, b = H-half), free
    dims are (row-within-half, w).  Both dh and dw shifts become plain
    free-dim offsets into an h/w-padded fp16 copy of feature2, and the
    DMA loads are fully contiguous per partition.
  - Inputs are cast fp32->fp16 during the DMA (SWDGE cast) so the 9 big
    element-wise products run in the DVE 2x perf mode.  A +1-element
    shifted copy of the f2 tile keeps the dw=+-1 reads 4-byte aligned.
  - The channel reduction (sum over C=64 within each half) runs on the
    otherwise-idle TensorEngine: each 128-column chunk of the product is
    the stationary operand of a matmul against a constant [128,2] 0/1
    mask whose columns select the halves.  PSUM output lands as
    [w, (h2, half)] = A_d transposed.  Three shifts share one PSUM bank
    (every matmul is its own start/stop group writing disjoint columns).
  - Everything is split into two row-half chunks so the DVE product
    chain starts as soon as the first half of the inputs has landed.
  - l2 normalization is folded into the softmax scores:
      score_d = 10 * A_d * rsqrt(|f1|^2) * rsqrt(|f2|^2 shifted)
    with rsqrt computed as exp(-0.5*ln(x)) (one ACT table set).
    Out-of-image shifted positions: zero rows in the f2 tile (dh) and
    zeroed edge rows of the w-shifted rsqrt maps (dw) force score=0 =>
    exp(0)=1, exactly the reference zero-padding behaviour.
  - Scores are bounded by |10| so softmax needs no max subtraction.
  - Output is written as [W, 2, H]; the host permutes back to [2, H, W].
"""

import numpy as np

B, C, H, W = 8, 64, 128, 128
N_CORES = 8
SOFTMAX_SCALE = 10.0

NH = 64          # rows per H-half
N1 = NH * W      # 8192 free elems per partition for f1 / products
RT = 68          # f2 tile rows (rows 1..66 hold data, 0/67 are pad)
N2 = RT * W      # 8704
HC = 32          # rows per chunk (half of NH)
NC1 = HC * W     # 4096, product chunk size

_CACHE = {}


def _build_program():
    import concourse.bass as bass
    import concourse.bacc as bacc
    import concourse.mybir as mybir
    from concourse.tile import TileContext
    from concourse.bass_utils import axon_active

    f32 = mybir.dt.float32
    f16 = mybir.dt.float16
    AF = mybir.ActivationFunctionType
    nc = bacc.Bacc(
        "TRN2",
        target_bir_lowering=False,
        debug=not axon_active(),
        num_devices=N_CORES,
    )

    f1d = nc.declare_dram_parameter("feature1", [C, H, W], f32, isOutput=False)
    f2d = nc.declare_dram_parameter("feature2", [C, H, W], f32, isOutput=False)
    # [W, 2, H]: transposed output, host permutes back to [2, H, W].
    outd = nc.declare_dram_parameter("flow", [W, 2, H], f32, isOutput=True)

    f1flat = f1d.rearrange("c h w -> c (h w)")               # [64, 16384]
    f2flat = f2d.rearrange("c h w -> c (h w)")               # [64, 16384]
    outv = outd.rearrange("w ch (b h) -> w (ch b h)", b=2)   # [128, 256]

    mask_np = np.zeros((128, 2), dtype=np.float16)
    mask_np[:64, 0] = 1.0
    mask_np[64:, 1] = 1.0
    maskd = nc.inline_tensor(mask_np, name="halfmask")

    red = dict(axis=mybir.AxisListType.X, op=mybir.AluOpType.add)
    DS = (1, 4, 7, 0, 2, 3, 5, 6, 8)  # dw=0 shifts first (no f2s needed)

    with TileContext(nc) as tc:
        with tc.tile_pool(name="main", bufs=1) as pool, \
             tc.tile_pool(name="prodp", bufs=3) as prodp, \
             tc.tile_pool(name="psum", bufs=1, space="PSUM") as psp:

            f1t = pool.tile([128, N1], f16)
            f2t = pool.tile([128, N2], f16)
            f2s = pool.tile([128, N2], f16)
            mskt = pool.tile([128, 2], f16)

            nc.sync.dma_start(out=mskt[:, :], in_=maskd[:, :])

            # zero the dh pad rows (r=1 half 0 / r=66 half 1) and the
            # corner-read rows 0/67; loads overwrite the data rows.
            nc.vector.memset(f2t[:, 0 : 2 * W], 0.0)
            nc.vector.memset(f2t[:, 66 * W : 68 * W], 0.0)

            # f2s is the +1-element shifted copy of f2t (keeps the dw=+-1
            # product reads 4-byte aligned for the DVE 2x mode).  It is
            # loaded straight from DRAM with the +1 shift applied on the
            # *source* side -- DRAM reads have no alignment penalty, while
            # a shifted SBUF->SBUF copy measured ~20x below line rate.
            nc.vector.memset(f2s[:, 0 : 2 * W], 0.0)
            nc.vector.memset(f2s[:, 66 * W - 1 : 68 * W], 0.0)

            # fp32 -> fp16 cast loads (SWDGE), chunked so the first-half
            # products can start before the second half lands.
            # f2 tile row r holds global row h = b*64 + r - 2.
            # chunk 1: f1 rows 0..31 per half, f2/f2s tile rows <= 36
            nc.gpsimd.dma_start(out=f1t[0:64, 0:NC1], in_=f1flat[:, 0:NC1])
            nc.gpsimd.dma_start(
                out=f1t[64:128, 0:NC1], in_=f1flat[:, N1 : N1 + NC1]
            )
            # b=0: h = 0..34 -> r = 2..36 ; b=1: h = 63..98 -> r = 1..36
            nc.gpsimd.dma_start(
                out=f2t[0:64, 2 * W : 37 * W], in_=f2flat[:, 0 : 35 * W]
            )
            nc.gpsimd.dma_start(
                out=f2t[64:128, 1 * W : 37 * W], in_=f2flat[:, 63 * W : 99 * W]
            )
            nc.gpsimd.dma_start(
                out=f2s[0:64, 2 * W : 37 * W], in_=f2flat[:, 1 : 35 * W + 1]
            )
            nc.gpsimd.dma_start(
                out=f2s[64:128, 1 * W : 37 * W],
                in_=f2flat[:, 63 * W + 1 : 99 * W + 1],
            )
            # chunk 2
            nc.gpsimd.dma_start(
                out=f1t[0:64, NC1:N1], in_=f1flat[:, NC1:N1]
            )
            nc.gpsimd.dma_start(
                out=f1t[64:128, NC1:N1], in_=f1flat[:, N1 + NC1 : 2 * N1]
            )
            # b=0: h = 35..64 -> r = 37..66 ; b=1: h = 99..127 -> r = 37..65
            nc.gpsimd.dma_start(
                out=f2t[0:64, 37 * W : 67 * W], in_=f2flat[:, 35 * W : 65 * W]
            )
            nc.gpsimd.dma_start(
                out=f2t[64:128, 37 * W : 66 * W], in_=f2flat[:, 99 * W : 128 * W]
            )
            nc.gpsimd.dma_start(
                out=f2s[0:64, 37 * W : 67 * W],
                in_=f2flat[:, 35 * W + 1 : 65 * W + 1],
            )
            nc.gpsimd.dma_start(
                out=f2s[64:128, 37 * W : 66 * W - 1],
                in_=f2flat[:, 99 * W + 1 : 128 * W],
            )

            # |f1|^2, |f2|^2 squares (ScalarE), chunked like the loads
            sq1 = pool.tile([128, N1], f16)
            sq2 = pool.tile([128, N2], f16)
            nc.scalar.activation(sq1[:, 0:NC1], f1t[:, 0:NC1], AF.Square)
            nc.scalar.activation(
                sq2[:, W : 37 * W], f2t[:, W : 37 * W], AF.Square
            )
            nc.scalar.activation(sq1[:, NC1:N1], f1t[:, NC1:N1], AF.Square)
            nc.scalar.activation(
                sq2[:, 37 * W : 67 * W], f2t[:, 37 * W : 67 * W], AF.Square
            )

            # channel reduction of the squares on the TensorEngine:
            # psR[w, (t, half)] = sum_c sq[(c,half), t*W + w].
            # R1 and R2 share one PSUM bank (disjoint columns).
            psR = psp.tile([128, 2, 66, 2], f32, tag="psR")
            psR1, psR2 = psR[:, 0, 0:64, :], psR[:, 1, :, :]
            for t in range(64):
                nc.tensor.matmul(
                    psR1[:, t, :],
                    lhsT=sq1[:, t * W : (t + 1) * W],
                    rhs=mskt[:, :],
                )
            for t in range(66):
                nc.tensor.matmul(
                    psR2[:, t, :],
                    lhsT=sq2[:, (1 + t) * W : (2 + t) * W],
                    rhs=mskt[:, :],
                )

            # rsqrt via exp(-0.5*ln(x)); bias keeps ln finite on zero rows
            rinv1 = pool.tile([128, 64, 2], f32)
            rinv2 = pool.tile([128, 66, 2], f32)
            tl1 = pool.tile([128, 64, 2], f32)
            tl2 = pool.tile([128, 66, 2], f32)
            epsb = pool.tile([128, 1], f32)
            nc.vector.memset(epsb[:, :], 1e-20)
            nc.scalar.activation(tl1[:, :, :], psR1[:, :, :], AF.Ln, bias=epsb[:, :])
            nc.scalar.activation(rinv1[:, :, :], tl1[:, :, :], AF.Exp, scale=-0.5)
            nc.scalar.activation(tl2[:, :, :], psR2[:, :, :], AF.Ln, bias=epsb[:, :])
            nc.scalar.activation(rinv2[:, :, :], tl2[:, :, :], AF.Exp, scale=-0.5)

            # w-shifted copies of rinv2 (partition shift via SBUF->SBUF DMA);
            # edge partitions stay 0 => score 0 => exp(0)=1 (reference pad).
            rec2m = pool.tile([128, 66, 2], f32)
            rec2p = pool.tile([128, 66, 2], f32)
            nc.vector.memset(rec2m[:, :, :], 0.0)
            nc.vector.memset(rec2p[:, :, :], 0.0)
            nc.sync.dma_start(out=rec2m[1:128, :, :], in_=rinv2[0:127, :, :])
            nc.sync.dma_start(out=rec2p[0:127, :, :], in_=rinv2[1:128, :, :])
            rsel = {-1: rec2m, 0: rinv2, 1: rec2p}

            # rr_d = rinv1 * rinv2(shifted); tiny DVE ops that fill gaps in
            # the product chain (GpSimd's Q7 stalled ~12us on these).
            rrs = {}
            for d in DS:
                dh, dw = d // 3 - 1, d % 3 - 1
                rr = pool.tile([128, 64, 2], f32, tag=f"rr{d}")
                nc.vector.tensor_mul(
                    rr[:, :, :], rinv1[:, :, :], rsel[dw][:, 1 + dh : 65 + dh, :]
                )
                rrs[d] = rr

            # 3 shifts share one PSUM bank; every matmul is its own group
            psA = {}
            for k in range(3):
                pa = psp.tile([128, 3, 64, 2], f32, tag=f"psA{k}")
                for j in range(3):
                    psA[DS[k * 3 + j]] = pa[:, j]

            def product_chunk(d, lo, hi):
                """prod[:, lo*W:hi*W] for shift d, then its PE reduction."""
                dh, dw = d // 3 - 1, d % 3 - 1
                if dw == 0:
                    src, base = f2t, (2 + dh) * W
                else:
                    src, base = f2s, (2 + dh) * W + dw - 1
                n = (hi - lo) * W
                prod = prodp.tile([128, NC1], f16, tag="prod")
                nc.vector.tensor_mul(
                    prod[:, 0:n],
                    f1t[:, lo * W : hi * W],
                    src[:, base + lo * W : base + hi * W],
                )
                for t in range(lo, hi):
                    nc.tensor.matmul(
                        psA[d][:, t, :],
                        lhsT=prod[:, (t - lo) * W : (t - lo + 1) * W],
                        rhs=mskt[:, :],
                    )

            s_all = pool.tile([128, 9, 64, 2], f32)
            e_all = pool.tile([128, 9, 64, 2], f32)
            for d in DS:
                product_chunk(d, 0, HC)
            for d in DS:
                product_chunk(d, HC, NH)
                nc.vector.tensor_mul(
                    s_all[:, d, :, :], psA[d][:, :, :], rrs[d][:, :, :]
                )
                nc.scalar.activation(
                    e_all[:, d, :, :], s_all[:, d, :, :], AF.Exp,
                    scale=SOFTMAX_SCALE,
                )

            # softmax-weighted displacement sums; outputs (b, h2)-major.
            # esum = fwm + fw0 + fwp (avoids a big 9-way strided reduce).
            esum = pool.tile([128, 2, 64], f32)
            fwm = pool.tile([128, 2, 64], f32)
            fw0 = pool.tile([128, 2, 64], f32)
            fwp = pool.tile([128, 2, 64], f32)
            fhm = pool.tile([128, 2, 64], f32)
            fhp = pool.tile([128, 2, 64], f32)
            ec = e_all.rearrange("p (a c) t m -> p c m t a", a=3)
            ea = e_all.rearrange("p (a c) t m -> p a m t c", a=3)
            nc.vector.tensor_reduce(fwm[:, :, :], ec[:, 0], **red)
            nc.vector.tensor_reduce(fw0[:, :, :], ec[:, 1], **red)
            nc.vector.tensor_reduce(fwp[:, :, :], ec[:, 2], **red)
            nc.vector.tensor_reduce(fhm[:, :, :], ea[:, 0], **red)
            nc.vector.tensor_reduce(fhp[:, :, :], ea[:, 2], **red)
            nc.vector.tensor_add(esum[:, :, :], fwm[:, :, :], fw0[:, :, :])
            nc.vector.tensor_add(esum[:, :, :], esum[:, :, :], fwp[:, :, :])

            resum = pool.tile([128, 2, 64], f32)
            nc.vector.reciprocal(resum[:, :, :], esum[:, :, :])
            nc.vector.tensor_sub(fwp[:, :, :], fwp[:, :, :], fwm[:, :, :])
            nc.vector.tensor_sub(fhp[:, :, :], fhp[:, :, :], fhm[:, :, :])

            # flows stored (w, ch, b, h2) so the DRAM write is contiguous
            flows = pool.tile([128, 2, 2, 64], f32)
            nc.vector.tensor_mul(flows[:, 0], fwp[:, :, :], resum[:, :, :])
            nc.vector.tensor_mul(flows[:, 1], fhp[:, :, :], resum[:, :, :])

            nc.sync.dma_start(out=outv, in_=flows[:, :, :, :])

    nc.compile()
    return nc


def _make_in_maps(f1: np.ndarray, f2: np.ndarray):
    return [{"feature1": f1[b], "feature2": f2[b]} for b in range(N_CORES)]


def kernel(feature1: np.ndarray, feature2: np.ndarray) -> np.ndarray:
    from concourse import bass_utils

    if "nc" not in _CACHE:
        _CACHE["nc"] = _build_program()
    nc = _CACHE["nc"]

    f1 = np.ascontiguousarray(np.asarray(feature1, dtype=np.float32))
    f2 = np.ascontiguousarray(np.asarray(feature2, dtype=np.float32))
    in_maps = _make_in_maps(f1, f2)
    res = bass_utils.run_bass_kernel_spmd(nc, in_maps, list(range(N_CORES)))
    # flow comes back [W, 2, H]; permute to [2, H, W]
    out = np.stack(
        [res.results[b]["flow"].transpose(1, 2, 0) for b in range(N_CORES)], axis=0
    )
    return np.ascontiguousarray(out.astype(np.float32))
